# revision 1
# baseline (speedup 1.0000x reference)
"""Longformer self-attention on 8 trn2 NeuronCores (Bass/Tile).

Sharding: 24 (batch, head) pairs -> 3 per core (batch = core//4, head group = core%4).

Algorithm (per core, per head): the reference multiplies dense scores by a
band mask (|j-i| <= 256) and then softmaxes over the FULL row, so masked
positions contribute exp(0)=1 to the denominator and v_j to the numerator.
Using T = sum_{j in span} exp(mask*s_ij) * [v_j | 1] over a 768-wide key span
per 256-query tile (zero-padded K/V at the sequence edges):
    numerator   = T[:64] + (sumv_all - sumv_span)
    denominator = T[64]  + (S_real   - n_real_span)
Both corrections are per-(head, tile) constants, computed on device from
prefix sums of V. So only the banded part of the score matrix is ever
computed/exponentiated; the out-of-band softmax mass is folded analytically.

Layouts: scores are computed transposed (S^T[j, q], j on partitions) so the
PV matmul can use P^T directly as the moving operand at full f32r rate.
Heads h0/h1 live at SBUF partitions 0-63/64-127 so their K=64 score matmuls
auto-pack into disjoint PE row groups and run concurrently.
"""

import math
import sys

import numpy as np

sys.path.insert(0, "/opt/trn_rl_repo")

import concourse.bass as bass
import concourse.mybir as mybir
import concourse.tile as tile
from concourse import bacc
from concourse.bass_utils import run_bass_kernel_spmd

B, S, HID = 2, 2048, 768
H, D = 12, 64
NCORES = 8
HPC = 3            # heads per core
QT = 256           # query tile size
PAD = 256          # key padding each side (= window)
SPAN = QT + 2 * PAD        # 768 key span per query tile
NCH = SPAN // 128          # 6 span chunks
SP = S + 2 * PAD           # 2560 padded key length
NVCH = SP // 128           # 20 V chunks
NT = S // QT               # 8 query tiles
F32 = mybir.dt.float32
F32R = mybir.dt.float32r
VSTR = 66
EXP = mybir.ActivationFunctionType.Exp

_CACHE: dict = {}


def _build_nc():
    nc = bacc.Bacc(
        trn_type="TRN2",
        target_bir_lowering=False,
        debug=False,
        num_devices=NCORES,
    )
    xt = nc.dram_tensor("xt", [HID, S], F32R, kind="ExternalInput").ap()
    wstack = nc.dram_tensor("wstack", [HID, 512], F32R, kind="ExternalInput").ap()
    bstack = nc.dram_tensor("bstack", [128, 4], F32, kind="ExternalInput").ap()
    wvt = nc.dram_tensor("wvt", [HID, 256], F32R, kind="ExternalInput").ap()
    bvrow = nc.dram_tensor("bvrow", [128, 256], F32, kind="ExternalInput").ap()
    masks = nc.dram_tensor("masks", [128, 4, QT], F32, kind="ExternalInput").ap()
    consts = nc.dram_tensor("consts", [128, 320], F32R, kind="ExternalInput").ap()
    yt = nc.dram_tensor("yt", [HPC * 64, S], F32, kind="ExternalOutput").ap()

    with tile.TileContext(nc) as tc:
        _kernel_body(tc, xt, wstack, bstack, wvt, bvrow, masks, consts, yt)
    nc.compile()
    return nc


def _kernel_body(tc, xt, wstack, bstack, wvt, bvrow, masks, consts, yt):
    nc = tc.nc
    with (
        tc.tile_pool(name="const", bufs=1) as constp,
        tc.tile_pool(name="acts", bufs=1) as actsp,
        tc.tile_pool(name="work", bufs=2) as workp,
    ):
        # ---- stage inputs in SBUF ----
        xt_sb = constp.tile([128, 6, S], F32R, tag="xt")
        for c in range(6):
            nc.sync.dma_start(xt_sb[:, c, :], xt[128 * c : 128 * c + 128, :])
        wst_sb = constp.tile([128, 6, 512], F32R, tag="wst")
        for c in range(6):
            nc.sync.dma_start(wst_sb[:, c, :], wstack[128 * c : 128 * c + 128, :])
        wvt_sb = constp.tile([128, 6, 256], F32R, tag="wvt")
        for c in range(6):
            nc.sync.dma_start(wvt_sb[:, c, :], wvt[128 * c : 128 * c + 128, :])
        bst_sb = constp.tile([128, 4], F32, tag="bst")
        nc.sync.dma_start(bst_sb[:, :], bstack[:, :])
        mask_sb = constp.tile([128, 4, QT], F32, tag="mask")
        nc.sync.dma_start(mask_sb[:, :, :], masks[:, :, :])
        bvb_sb = constp.tile([128, 256], F32, tag="bvb")
        nc.sync.dma_start(bvb_sb[:, :], bvrow[:, :])
        consts_sb = constp.tile([128, 320], F32R, tag="consts")
        nc.sync.dma_start(consts_sb[:, :], consts[:, :])
        ones_col = consts_sb[:, 0:2]
        ones_row = consts_sb[0:1, 0:64]
        zeros256 = consts[:, 64:320]  # dram zeros for f32r pad fills

        # ---- activations ----
        qa = actsp.tile([128, S], F32R, tag="qa")    # Q head0 | Q head1
        qb = actsp.tile([128, S], F32R, tag="qb")    # Q head2 | junk
        ka = actsp.tile([128, SP], F32R, tag="ka")   # K head0 | K head1 (padded)
        kb = actsp.tile([128, SP], F32R, tag="kb")   # K head2 | junk (padded)
        vt = actsp.tile([128, NVCH, 256], F32R, tag="vt")  # V chunks, padded
        corr_sb = actsp.tile([66, HPC, NT], F32, tag="corr")

        for kt in (ka, kb):
            nc.sync.dma_start(kt[:, 0:PAD], zeros256)
            nc.sync.dma_start(kt[:, PAD + S : SP], zeros256)
        for vc in (0, 1, NVCH - 2, NVCH - 1):
            nc.sync.dma_start(vt[:, vc, :], zeros256)

        # ---- QK projections: psum[m,n] = wstack_chunk^T @ xt ----
        qk_dst = [(qa, 0), (ka, PAD), (qb, 0), (kb, PAD)]
        with tc.tile_pool(name="pps", bufs=3, space="PSUM") as pps:
            for m in range(4):
                dstt, coff = qk_dst[m]
                for n in range(4):
                    ps = pps.tile([128, 512], F32, tag="ps")
                    for kc in range(6):
                        nc.tensor.matmul(
                            ps[:, :],
                            lhsT=wst_sb[:, kc, 128 * m : 128 * m + 128],
                            rhs=xt_sb[:, kc, 512 * n : 512 * n + 512],
                            start=(kc == 0),
                            stop=(kc == 5),
                        )
                    nc.vector.tensor_scalar_add(
                        dstt[:, coff + 512 * n : coff + 512 * n + 512],
                        ps[:, :],
                        bst_sb[:, m : m + 1],
                    )

        # ---- V projection (natural layout, 65-col per head incl. ones) ----
        with tc.tile_pool(name="vps", bufs=3, space="PSUM") as vps:
            for sc in range(16):
                ps = vps.tile([128, 256], F32, tag="vp")
                for kc in range(6):
                    nc.tensor.matmul(
                        ps[:, :],
                        lhsT=xt_sb[:, kc, 128 * sc : 128 * sc + 128],
                        rhs=wvt_sb[:, kc, :],
                        start=(kc == 0),
                        stop=(kc == 5),
                    )
                nc.vector.tensor_add(vt[:, sc + 2, :], ps[:, :], bvb_sb[:, :])

        # ---- prefix sums of V chunk-pairs -> per-(head, tile) corrections ----
        with tc.tile_pool(name="prps", bufs=3, space="PSUM") as prps:
            for h in range(HPC):
                pp = prps.tile([66, 20], F32, tag="pp")
                for k in range(10):
                    for e in range(2):
                        nc.tensor.matmul(
                            pp[:, 2 * k : 2 * k + 2],
                            lhsT=vt[:, 2 * k + e, VSTR * h : VSTR * h + VSTR],
                            rhs=ones_col[:, :],
                            start=(e == 0),
                            stop=(e == 1),
                        )
                pfx = workp.tile([66, 10], F32, tag="pfx")
                nc.vector.tensor_copy(pfx[:, :], pp[:, 0:20:2])
                for k in range(1, 10):
                    nc.vector.tensor_add(
                        pfx[:, k : k + 1], pfx[:, k : k + 1], pfx[:, k - 1 : k]
                    )
                # corr(t) = total - prefix[2t+5] + prefix[2t-1]
                for t in range(NT):
                    nc.vector.tensor_sub(
                        corr_sb[:, h, t : t + 1], pfx[:, 9:10], pfx[:, t + 2 : t + 3]
                    )
                    if t > 0:
                        nc.vector.tensor_add(
                            corr_sb[:, h, t : t + 1],
                            corr_sb[:, h, t : t + 1],
                            pfx[:, t - 1 : t],
                        )

        # ---- banded attention ----
        qk_srcs = [(qa, ka, 0), (qa, ka, 64), (qb, kb, 0)]
        with (
            tc.tile_pool(name="sps", bufs=2, space="PSUM") as sps,
            tc.tile_pool(name="cps", bufs=2, space="PSUM") as cps,
            tc.tile_pool(name="psb", bufs=3) as psb,
            tc.tile_pool(name="osb", bufs=4) as osb,
        ):
            for t in range(NT):
                for h in range(HPC):
                    qt_, kt_, base = qk_srcs[h]
                    sp_ = sps.tile([128, NCH, QT], F32, tag="sp")
                    for c in range(NCH):
                        nc.tensor.matmul(
                            sp_[:, c, :],
                            lhsT=kt_[
                                base : base + 64,
                                QT * t + 128 * c : QT * t + 128 * c + 128,
                            ],
                            rhs=qt_[base : base + 64, QT * t : QT * t + QT].bitcast(
                                F32R
                            ),
                            start=True,
                            stop=True,
                        )
                    for slot, c in enumerate((0, 1, 4, 5)):
                        nc.vector.tensor_mul(
                            sp_[:, c, :], sp_[:, c, :], mask_sb[:, slot, :]
                        )
                    pt = psb.tile([128, NCH, QT], F32R, tag="pt")
                    nc.scalar.activation(pt[:, :, :], sp_[:, :, :], EXP)
                    cp = cps.tile([66, 2, QT], F32, tag="cp")
                    for c in range(NCH):
                        nc.tensor.matmul(
                            cp[:, 0, :],
                            lhsT=vt[:, 2 * t + c, VSTR * h : VSTR * h + VSTR],
                            rhs=pt[:, c, :],
                            start=(c == 0),
                            stop=(c == NCH - 1),
                        )
                    ctx_sb = osb.tile([66, QT], F32, tag="ctx")
                    nc.vector.tensor_scalar_add(
                        ctx_sb[:, :], cp[:, 0, :], corr_sb[:, h, t : t + 1]
                    )
                    rec = osb.tile([1, QT], F32R, tag="rec")
                    with nc.allow_low_precision(reason="f32r denominators"):
                        nc.vector.reciprocal(rec[:, :], ctx_sb[64:65, :])
                    # broadcast rec across 64 partitions via a K=1 matmul
                    nc.tensor.matmul(
                        cp[0:64, 1, :],
                        lhsT=ones_row[:, :],
                        rhs=rec[:, :],
                        start=True,
                        stop=True,
                    )
                    ot = osb.tile([64, QT], F32, tag="ot")
                    nc.vector.tensor_mul(ot[:, :], ctx_sb[0:64, :], cp[0:64, 1, :])
                    nc.sync.dma_start(
                        yt[64 * h : 64 * h + 64, QT * t : QT * t + QT], ot[:, :]
                    )


def _round_f32r(a: np.ndarray) -> np.ndarray:
    """Round fp32 to E8M11 (fp32r) with round-to-nearest-even."""
    u = np.ascontiguousarray(a, np.float32).view(np.uint32).copy()
    rb = (u >> 12) & 1
    u += np.uint32(0x7FF) + rb
    u &= np.uint32(0xFFFFF000)
    return u.view(np.float32)


def _band_masks() -> np.ndarray:
    # mask[c][j, q]: in-band iff |128c - 256 + j - q| <= 256, for c in {0,1,4,5}
    jj = np.arange(128)[:, None]
    qq = np.arange(QT)[None, :]
    out = np.zeros((128, 4, QT), np.float32)
    for slot, c in enumerate((0, 1, 4, 5)):
        rel = 128 * c - 256 + jj - qq
        out[:, slot, :] = (np.abs(rel) <= 256).astype(np.float32)
    return out


def _make_in_maps(hs, Wq, bq, Wk, bk, Wv, bv):
    masks = _band_masks()
    sc = 1.0 / math.sqrt(D)
    in_maps = []
    for core in range(NCORES):
        b = core // 4
        hg = core % 4
        heads = [HPC * hg + i for i in range(HPC)]
        xt = np.ascontiguousarray(hs[b].T)
        wstack = np.zeros((HID, 512), np.float32)
        bstack = np.zeros((128, 4), np.float32)
        h0, h1, h2 = heads
        wstack[:, 0:64] = Wq[64 * h0 : 64 * h0 + 64, :].T * sc
        wstack[:, 64:128] = Wq[64 * h1 : 64 * h1 + 64, :].T * sc
        wstack[:, 128:192] = Wk[64 * h0 : 64 * h0 + 64, :].T
        wstack[:, 192:256] = Wk[64 * h1 : 64 * h1 + 64, :].T
        wstack[:, 256:320] = Wq[64 * h2 : 64 * h2 + 64, :].T * sc
        wstack[:, 384:448] = Wk[64 * h2 : 64 * h2 + 64, :].T
        bstack[0:64, 0] = bq[64 * h0 : 64 * h0 + 64] * sc
        bstack[64:128, 0] = bq[64 * h1 : 64 * h1 + 64] * sc
        bstack[0:64, 1] = bk[64 * h0 : 64 * h0 + 64]
        bstack[64:128, 1] = bk[64 * h1 : 64 * h1 + 64]
        bstack[0:64, 2] = bq[64 * h2 : 64 * h2 + 64] * sc
        bstack[0:64, 3] = bk[64 * h2 : 64 * h2 + 64]
        wvt = np.zeros((HID, 256), np.float32)
        bvrow = np.zeros((1, 256), np.float32)
        for i, h in enumerate(heads):
            wvt[:, VSTR * i : VSTR * i + 64] = Wv[64 * h : 64 * h + 64, :].T
            bvrow[0, VSTR * i : VSTR * i + 64] = bv[64 * h : 64 * h + 64]
            bvrow[0, VSTR * i + 64] = 1.0
        bvrow = np.ascontiguousarray(np.broadcast_to(bvrow, (128, 256)))
        consts = np.zeros((128, 320), np.float32)
        consts[:, 0:64] = 1.0
        in_maps.append(
            {
                "consts": consts,
                "xt": _round_f32r(xt),
                "wstack": _round_f32r(wstack),
                "bstack": bstack,
                "wvt": _round_f32r(wvt),
                "bvrow": bvrow,
                "masks": masks,
            }
        )
    return in_maps


def kernel(hidden_states, Wq, bq, Wk, bk, Wv, bv):
    hs = np.ascontiguousarray(np.asarray(hidden_states, np.float32))
    Wq = np.asarray(Wq, np.float32)
    Wk = np.asarray(Wk, np.float32)
    Wv = np.asarray(Wv, np.float32)
    bq = np.asarray(bq, np.float32)
    bk = np.asarray(bk, np.float32)
    bv = np.asarray(bv, np.float32)

    if "nc" not in _CACHE:
        _CACHE["nc"] = _build_nc()
    nc = _CACHE["nc"]

    in_maps = _make_in_maps(hs, Wq, bq, Wk, bk, Wv, bv)
    res = run_bass_kernel_spmd(nc, in_maps, list(range(NCORES)))
    out = np.zeros((B, S, HID), np.float32)
    for core in range(NCORES):
        ytc = res.results[core]["yt"]
        b = core // 4
        hg = core % 4
        out[b, :, 192 * hg : 192 * hg + 192] = ytc.T
    return out


if __name__ == "__main__":
    np.random.seed(0)
    hs = np.random.randn(B, S, HID).astype(np.float32)
    z = np.zeros((HID,), np.float32)
    w = (np.random.randn(HID, HID) / math.sqrt(HID)).astype(np.float32)
    out = kernel(hs, w, z, w, z, w, z)
    print(out.shape, out.dtype)

